# revision 3
# baseline (speedup 1.0000x reference)
import sys, os
import numpy as np
from contextlib import ExitStack

sys.path.insert(0, "/opt/trn_rl_repo")

import concourse.bass as bass
import concourse.mybir as mybir
from concourse import tile, bacc
from concourse.bass_utils import run_bass_kernel_spmd

F32 = mybir.dt.float32

N = 1_000_000
H = 128
G = 32768
CORES = 8
NS = 125_440           # per-core padded shard (980 tiles of 128)
CH = 512               # nodes per chunk
NCHUNK = NS // CH      # 245
SUB = CH // 128        # 4 sub-tiles of 128 nodes
WIN = 64               # graph-window width per chunk

_CACHE = {}


def _build_module():
    nc = bacc.Bacc()
    x = nc.declare_dram_parameter("x", [NS, H], F32, isOutput=False)
    colT = nc.declare_dram_parameter("colT", [128, NCHUNK * SUB], F32, isOutput=False)
    W1 = nc.declare_dram_parameter("W1", [H, 64], F32, isOutput=False)
    b1c = nc.declare_dram_parameter("b1c", [64, 1], F32, isOutput=False)
    W2 = nc.declare_dram_parameter("W2", [64, 1], F32, isOutput=False)
    b2c = nc.declare_dram_parameter("b2c", [128, 1], F32, isOutput=False)
    iota = nc.declare_dram_parameter("iota", [128, WIN], F32, isOutput=False)
    ident = nc.declare_dram_parameter("ident", [128, 128], F32, isOutput=False)
    e_out = nc.declare_dram_parameter("e_out", [128, NCHUNK * SUB], F32, isOutput=True)
    pool_out = nc.declare_dram_parameter("pool_out", [NCHUNK, WIN, H], F32, isOutput=True)

    x_v = x.rearrange("(c s p) h -> c p s h", s=SUB, p=128)

    with ExitStack() as ctx:
        tc = ctx.enter_context(tile.TileContext(nc))
        const = ctx.enter_context(tc.tile_pool(name="const", bufs=1))
        W1_sb = const.tile([H, 64], F32, tag="w1")
        nc.sync.dma_start(W1_sb[:], W1[:])
        b1_sb = const.tile([64, 1], F32, tag="b1")
        nc.sync.dma_start(b1_sb[:], b1c[:])
        W2_sb = const.tile([64, 1], F32, tag="w2")
        nc.sync.dma_start(W2_sb[:], W2[:])
        b2_sb = const.tile([128, 1], F32, tag="b2")
        nc.sync.dma_start(b2_sb[:], b2c[:])
        iota_sb = const.tile([128, WIN], F32, tag="iota")
        nc.sync.dma_start(iota_sb[:], iota[:])
        id_sb = const.tile([128, 128], F32, tag="ident")
        nc.sync.dma_start(id_sb[:], ident[:])
        col_sb = const.tile([128, NCHUNK * SUB], F32, tag="col")
        nc.sync.dma_start(col_sb[:], colT[:])
        e_all = const.tile([128, NCHUNK * SUB], F32, tag="eall")

        xn_pool = ctx.enter_context(tc.tile_pool(name="xn", bufs=3))
        xt_pool = ctx.enter_context(tc.tile_pool(name="xt", bufs=2))
        ht_pool = ctx.enter_context(tc.tile_pool(name="ht", bufs=2))
        mk_pool = ctx.enter_context(tc.tile_pool(name="mk", bufs=8))
        po_pool = ctx.enter_context(tc.tile_pool(name="po", bufs=2))
        pxt = ctx.enter_context(tc.tile_pool(name="pxt", bufs=2, space="PSUM"))
        ph = ctx.enter_context(tc.tile_pool(name="ph", bufs=2, space="PSUM"))
        ps = ctx.enter_context(tc.tile_pool(name="ps", bufs=2, space="PSUM"))
        pp = ctx.enter_context(tc.tile_pool(name="pp", bufs=2, space="PSUM"))

        for c in range(NCHUNK):
            xn = xn_pool.tile([128, CH], F32, tag="xn")
            nc.sync.dma_start(
                xn[:].rearrange("p (s h) -> p s h", s=SUB), x_v[c]
            )
            # transpose each 128-node subtile -> xT [H, node]
            xt = xt_pool.tile([128, CH], F32, tag="xt")
            for s in range(SUB):
                tp = pxt.tile([128, 128], F32, tag="tp")
                nc.tensor.transpose(tp[:], xn[:, s * 128:(s + 1) * 128], id_sb[:])
                nc.vector.tensor_copy(xt[:, s * 128:(s + 1) * 128], tp[:])
            # hT = tanh(W1^T xT + b1): [64, CH]
            hp = ph.tile([64, CH], F32, tag="hp")
            nc.tensor.matmul(hp[:], W1_sb[:], xt[:], start=True, stop=True)
            ht = ht_pool.tile([64, CH], F32, tag="ht")
            nc.scalar.activation(
                ht[:], hp[:], mybir.ActivationFunctionType.Tanh, bias=b1_sb[:, 0:1]
            )
            # s per subtile: [128 node, 1] = ht_sub^T @ W2 ; e = exp(s + b2)
            sp = ps.tile([128, SUB], F32, tag="sp")
            for s in range(SUB):
                nc.tensor.matmul(
                    sp[:, s:s + 1], ht[:, s * 128:(s + 1) * 128], W2_sb[:],
                    start=True, stop=True,
                )
            for s in range(SUB):
                nc.scalar.activation(
                    e_all[:, c * SUB + s: c * SUB + s + 1], sp[:, s:s + 1],
                    mybir.ActivationFunctionType.Exp, bias=b2_sb[:, 0:1],
                )
            # mask = (iota == col) * e ; pooled partial = sum_s mask_s^T @ x_s
            pl = pp.tile([WIN, H], F32, tag="pl")
            for s in range(SUB):
                mk = mk_pool.tile([128, WIN], F32, tag="mk")
                nc.vector.tensor_scalar(
                    mk[:], iota_sb[:],
                    col_sb[:, c * SUB + s: c * SUB + s + 1],
                    e_all[:, c * SUB + s: c * SUB + s + 1],
                    op0=mybir.AluOpType.is_equal, op1=mybir.AluOpType.mult,
                )
                nc.tensor.matmul(
                    pl[:], mk[:], xn[:, s * 128:(s + 1) * 128],
                    start=(s == 0), stop=(s == SUB - 1),
                )
            po = po_pool.tile([WIN, H], F32, tag="po")
            nc.vector.tensor_copy(po[:], pl[:])
            nc.sync.dma_start(pool_out[c], po[:])

        nc.sync.dma_start(e_out[:], e_all[:])
    nc.finalize()
    return nc


def kernel(x, batch, W1, b1, W2, b2, num_graphs):
    x = np.asarray(x, dtype=np.float32)
    batch = np.asarray(batch).astype(np.int64)
    W1 = np.asarray(W1, dtype=np.float32)
    b1 = np.asarray(b1, dtype=np.float32)
    W2 = np.asarray(W2, dtype=np.float32)
    b2 = np.asarray(b2, dtype=np.float32)
    num_graphs = int(num_graphs)
    n = x.shape[0]

    if "nc" not in _CACHE:
        _CACHE["nc"] = _build_module()
    nc = _CACHE["nc"]

    total = NS * CORES
    batch_pad = np.concatenate([batch, np.zeros(total - n, dtype=np.int64)])
    # per-chunk window base = graph id of first node in chunk
    bases = batch_pad.reshape(CORES * NCHUNK, CH)[:, 0].copy()  # [CORES*NCHUNK]
    col = batch_pad - np.repeat(bases, CH)
    col[n:] = WIN + 63  # padding nodes never match the iota window
    assert col.max() < 128 and (col[:n].max() < WIN), "window overflow"

    iota_np = np.broadcast_to(
        np.arange(WIN, dtype=np.float32), (128, WIN)
    ).copy()
    ident_np = np.eye(128, dtype=np.float32)
    b1c = b1.reshape(64, 1).copy()
    b2c = np.full((128, 1), float(b2[0]), dtype=np.float32)

    in_maps = []
    for c in range(CORES):
        lo, hi = c * NS, min((c + 1) * NS, n)
        xs = x[lo:hi]
        if xs.shape[0] < NS:
            xs = np.concatenate(
                [xs, np.zeros((NS - xs.shape[0], H), dtype=np.float32)]
            )
        colT = (
            col[c * NS:(c + 1) * NS]
            .reshape(NCHUNK * SUB, 128)
            .T.astype(np.float32)
            .copy()
        )
        in_maps.append({
            "x": np.ascontiguousarray(xs),
            "colT": colT,
            "W1": W1, "b1c": b1c, "W2": W2, "b2c": b2c,
            "iota": iota_np, "ident": ident_np,
        })

    res = run_bass_kernel_spmd(nc, in_maps, list(range(CORES))).results

    # host: gather e, combine pooled partials, normalize
    e_full = np.empty(total, dtype=np.float32)
    pooled = np.zeros((num_graphs + WIN, H), dtype=np.float64)
    for c in range(CORES):
        e_full[c * NS:(c + 1) * NS] = (
            res[c]["e_out"].T.reshape(NS)
        )
        part = res[c]["pool_out"]  # [NCHUNK, WIN, H]
        for k in range(NCHUNK):
            b0 = bases[c * NCHUNK + k]
            pooled[b0:b0 + WIN] += part[k]
    e_full = e_full[:n]

    # segment sums of e via f64 cumsum (safe for empty segments)
    cs = np.concatenate([[0.0], np.cumsum(e_full, dtype=np.float64)])
    starts = np.searchsorted(batch, np.arange(num_graphs))
    ends = np.concatenate([starts[1:], [n]])
    denom = (cs[ends] - cs[starts]).astype(np.float64)  # [G]

    attn = (e_full / denom[batch]).astype(np.float32)
    safe = np.where(denom > 0, denom, 1.0)
    x_pooled = (pooled[:num_graphs] / safe[:, None]).astype(np.float32)
    return x_pooled, attn
